# revision 1
# baseline (speedup 1.0000x reference)
"""Bidirectional co-attention kernel for Trainium2 (8 NeuronCores).

Problem: X, Y: (16, 2048, 300) f32.
  S_b = X_b @ Y_b^T                      (2048 x 2048 per batch)
  A1 = softmax_rows(S) @ Y * X
  A2 = softmax_rows(S^T) @ X * Y
  out = concat(A1, A2, axis=1)           -> (16, 4096, 300) f32

Sharding: data-parallel over batch, 2 batches per core, no cross-core comm.

Per-core algorithm (per batch):
  - S strips [128n x 2048m] and T = S^T strips [128m x 2048n] on TensorE in
    float32r (full-rate, ~11-bit-mantissa-accurate).
  - E = exp(S - 100) on ScalarE straight out of PSUM into bf16 SBUF, with
    accum_out giving the softmax denominators s1[n], s2[m] for free.
    (Fixed shift instead of row-max: scores are N(0, 300); max |S| ~ 95 so
    exp(S-100) never overflows, and row maxes are >> -80 so no fatal
    underflow. Normalization cancels the shift exactly.)
  - O1[n,:] = sum_m E_T[m, n-slice]^T @ Y[m,:] (bf16 matmul, f32 PSUM accum),
    then A1 = O1 * X * (1/s1) on VectorE.  Direction 2 symmetric from E_S.
"""

import numpy as np
import ml_dtypes

B, N, D = 16, 2048, 300
NCORES = 8
BPC = B // NCORES  # batches per core
NT = N // 128  # 16 row-tiles
KT = 3  # k-tiles over D (128+128+44, zero-padded to 384)
GSHIFT = -100.0
NBLK = 512  # moving-dim block for the score matmuls

_BF16 = ml_dtypes.bfloat16

_cache: dict[str, object] = {}


def _build():
    import concourse.bacc as bacc
    import concourse.mybir as mybir
    import concourse.tile as tile

    nc = bacc.Bacc("TRN2", target_bir_lowering=False, debug=False, num_devices=NCORES)

    f32 = mybir.dt.float32
    f32r = mybir.dt.float32r
    bf16 = mybir.dt.bfloat16

    xt_d = nc.dram_tensor("xt", [BPC, 128, KT * N], f32r, kind="ExternalInput")
    yt_d = nc.dram_tensor("yt", [BPC, 128, KT * N], f32r, kind="ExternalInput")
    xn_d = nc.dram_tensor("xn", [BPC, N, D], bf16, kind="ExternalInput")
    yn_d = nc.dram_tensor("yn", [BPC, N, D], bf16, kind="ExternalInput")
    out_d = nc.dram_tensor("out", [BPC, 2 * N, D], f32, kind="ExternalOutput")

    with tile.TileContext(nc) as tc:
        with (
            tc.tile_pool(name="const", bufs=1) as constp,
            tc.tile_pool(name="io", bufs=1) as io,
            tc.tile_pool(name="nat", bufs=2 * NT) as nat,
            tc.tile_pool(name="epool", bufs=NT) as epool,
            tc.tile_pool(name="stats", bufs=4 * NT) as stats,
            tc.tile_pool(name="abuf", bufs=4) as abuf,
            tc.tile_pool(name="psum", bufs=2, space="PSUM") as psum,
        ):
            bias_t = constp.tile([128, 1], f32, name="bias_t")
            nc.vector.memset(bias_t[:], GSHIFT)

            for b in range(BPC):
                xt_t = io.tile([128, KT * N], f32r, tag="xt", name=f"xt{b}")
                yt_t = io.tile([128, KT * N], f32r, tag="yt", name=f"yt{b}")
                nc.sync.dma_start(xt_t[:], xt_d.ap()[b])
                nc.sync.dma_start(yt_t[:], yt_d.ap()[b])

                xn_t = []
                yn_t = []
                for i in range(NT):
                    xi = nat.tile([128, D], bf16, tag="nat", name=f"xn{b}_{i}")
                    yi = nat.tile([128, D], bf16, tag="nat", name=f"yn{b}_{i}")
                    nc.sync.dma_start(xi[:], xn_d.ap()[b, i * 128 : (i + 1) * 128, :])
                    nc.sync.dma_start(yi[:], yn_d.ap()[b, i * 128 : (i + 1) * 128, :])
                    xn_t.append(xi)
                    yn_t.append(yi)

                es_t, et_t, s1_t, s2_t = [], [], [], []
                # ---- phase 1: score strips + exp ----
                for i in range(NT):
                    for which in range(2):  # 0: S strip (n-tile i), 1: T strip
                        lt, rt = (xt_t, yt_t) if which == 0 else (yt_t, xt_t)
                        sp = psum.tile(
                            [128, N], f32, tag="strip", name=f"sp{b}_{i}_{which}"
                        )
                        for k in range(KT):
                            lhsT = lt[:, k * N + i * 128 : k * N + i * 128 + 128]
                            for j in range(N // NBLK):
                                nc.tensor.matmul(
                                    sp[:, j * NBLK : (j + 1) * NBLK],
                                    lhsT,
                                    rt[:, k * N + j * NBLK : k * N + (j + 1) * NBLK],
                                    start=(k == 0),
                                    stop=(k == KT - 1),
                                )
                        ei = epool.tile(
                            [128, N],
                            bf16,
                            tag=("es" if which == 0 else "et"),
                            name=f"e{b}_{i}_{which}",
                        )
                        si = stats.tile(
                            [128, 1], f32, tag="stats", name=f"s{b}_{i}_{which}"
                        )
                        nc.scalar.activation(
                            out=ei[:],
                            in_=sp[:],
                            func=mybir.ActivationFunctionType.Exp,
                            bias=bias_t[:],
                            scale=1.0,
                            accum_out=si[:],
                        )
                        (es_t if which == 0 else et_t).append(ei)
                        (s1_t if which == 0 else s2_t).append(si)

                # ---- phase 2: PV matmuls + epilogue ----
                for i in range(NT):
                    for which in range(2):  # 0: A1 (rows i*128), 1: A2 (rows N+i*128)
                        et = et_t if which == 0 else es_t
                        rn = yn_t if which == 0 else xn_t
                        mult_n = xn_t[i] if which == 0 else yn_t[i]
                        si = (s1_t if which == 0 else s2_t)[i]
                        op = psum.tile(
                            [128, D], f32, tag="strip", name=f"o{b}_{i}_{which}"
                        )
                        for m in range(NT):
                            nc.tensor.matmul(
                                op[:],
                                et[m][:, i * 128 : (i + 1) * 128],
                                rn[m][:],
                                start=(m == 0),
                                stop=(m == NT - 1),
                            )
                        ri = stats.tile(
                            [128, 1], f32, tag="stats", name=f"r{b}_{i}_{which}"
                        )
                        nc.vector.reciprocal(ri[:], si[:])
                        ai = abuf.tile([128, D], f32, tag="a", name=f"a{b}_{i}_{which}")
                        nc.vector.tensor_mul(ai[:], op[:], mult_n[:])
                        nc.vector.tensor_scalar_mul(ai[:], ai[:], ri[:])
                        row0 = which * N + i * 128
                        nc.sync.dma_start(out_d.ap()[b, row0 : row0 + 128, :], ai[:])

    nc.compile()
    return nc


def _prep(arr_f32: np.ndarray) -> tuple[np.ndarray, np.ndarray]:
    """arr [Bc, N, D] f32 -> (packed-transposed f32 [Bc,128,KT*N], bf16 natural)."""
    bc = arr_f32.shape[0]
    t = np.zeros((bc, KT * 128, N), np.float32)
    t[:, :D, :] = arr_f32.transpose(0, 2, 1)
    t = t.reshape(bc, KT, 128, N).transpose(0, 2, 1, 3).reshape(bc, 128, KT * N)
    return np.ascontiguousarray(t), np.ascontiguousarray(arr_f32.astype(_BF16))


def kernel(X, Y, _trace=False, _trace_kwargs=None):
    from concourse.bass_utils import run_bass_kernel_spmd

    X = np.asarray(X, dtype=np.float32)
    Y = np.asarray(Y, dtype=np.float32)
    assert X.shape == (B, N, D) and Y.shape == (B, N, D)

    if "nc" not in _cache:
        _cache["nc"] = _build()
    nc = _cache["nc"]

    in_maps = []
    for c in range(NCORES):
        sl = slice(c * BPC, (c + 1) * BPC)
        xt, xn = _prep(X[sl])
        yt, yn = _prep(Y[sl])
        in_maps.append({"xt": xt, "yt": yt, "xn": xn, "yn": yn})

    res = run_bass_kernel_spmd(
        nc,
        in_maps,
        core_ids=list(range(NCORES)),
        trace=_trace,
        **(_trace_kwargs or {}),
    )
    _cache["last_results"] = res

    out = np.empty((B, 2 * N, D), np.float32)
    for c in range(NCORES):
        out[c * BPC : (c + 1) * BPC] = res.results[c]["out"]
    return out


# revision 4
# speedup vs baseline: 1.0199x; 1.0199x over previous
"""Bidirectional co-attention kernel for Trainium2 (8 NeuronCores).

Problem: X, Y: (16, 2048, 300) f32.
  S_b = X_b @ Y_b^T                      (2048 x 2048 per batch)
  A1 = softmax_rows(S) @ Y * X
  A2 = softmax_rows(S^T) @ X * Y
  out = concat(A1, A2, axis=1)           -> (16, 4096, 300) f32

Sharding: data-parallel over batch, 2 batches per core, no cross-core comm.

Per-core algorithm (per batch):
  Phase A: S strips [128n x 2048m] on TensorE in float32r (full-rate,
    ~11-bit-mantissa accurate).  E_S = exp(S - 100) on ScalarE straight out
    of PSUM into bf16 SBUF; accum_out gives row sums s1[n] for free.
    (Fixed shift instead of row-max: scores are N(0, 300) so max |S| ~ 95;
    exp(S-100) never overflows and row maxes are far above the underflow
    cliff.  Normalization cancels the shift exactly.)
  Phase B: E_T = E_S^T via PE-mode transposes (128x128 bf16), 8 tiles packed
    per PSUM bank, evicted by VectorE tensor_scalar copy whose accum_out
    yields the column sums s2[m].
  Phase C: O1[n,:] = sum_m E_T[m, n-slice]^T @ Y[m,:] (bf16 matmul, f32 PSUM
    accum), A1 = O1 * X * (1/s1) on VectorE.  Direction 2 symmetric from E_S.
"""

import numpy as np
import ml_dtypes

B, N, D = 16, 2048, 300
NCORES = 8
BPC = B // NCORES  # batches per core
NT = N // 128  # 16 row-tiles
KT = 3  # k-tiles over D (128+128+44, zero-padded to 384)
GSHIFT = -100.0
NBLK = 512  # moving-dim block for the score matmuls

_BF16 = ml_dtypes.bfloat16

_cache: dict[str, object] = {}


def _build():
    import concourse.bacc as bacc
    import concourse.mybir as mybir
    import concourse.tile as tile
    from concourse.masks import make_identity

    nc = bacc.Bacc("TRN2", target_bir_lowering=False, debug=False, num_devices=NCORES)

    f32 = mybir.dt.float32
    f32r = mybir.dt.float32r
    bf16 = mybir.dt.bfloat16

    xt_d = nc.dram_tensor("xt", [BPC, KT, 128, N], f32r, kind="ExternalInput")
    yt_d = nc.dram_tensor("yt", [BPC, KT, 128, N], f32r, kind="ExternalInput")
    xn_d = nc.dram_tensor("xn", [BPC, N, D], bf16, kind="ExternalInput")
    yn_d = nc.dram_tensor("yn", [BPC, N, D], bf16, kind="ExternalInput")
    out_d = nc.dram_tensor("out", [BPC, 2 * N, D], f32, kind="ExternalOutput")

    with tile.TileContext(nc) as tc:
        with (
            tc.tile_pool(name="const", bufs=1) as constp,
            tc.tile_pool(name="io", bufs=KT) as io,
            tc.tile_pool(name="nat", bufs=2 * NT) as nat,
            tc.tile_pool(name="epool", bufs=2 * NT) as epool,
            tc.tile_pool(name="stats", bufs=8 * NT) as stats,
            tc.tile_pool(name="abuf", bufs=4) as abuf,
            tc.tile_pool(name="psum", bufs=2, space="PSUM") as psum,
        ):
            bias_t = constp.tile([128, 1], f32, name="bias_t")
            nc.vector.memset(bias_t[:], GSHIFT)
            ident = constp.tile([128, 128], bf16, name="ident")
            make_identity(nc, ident[:])

            for b in range(BPC):
                xt_t, yt_t = [], []
                for k in range(KT):
                    xk = io.tile([128, N], f32r, tag="xt", name=f"xt{b}_{k}")
                    yk = io.tile([128, N], f32r, tag="yt", name=f"yt{b}_{k}")
                    nc.sync.dma_start(yk[:], yt_d.ap()[b, k])
                    nc.sync.dma_start(xk[:], xt_d.ap()[b, k])
                    xt_t.append(xk)
                    yt_t.append(yk)

                # ---- phase A: S strips + exp ----
                es_t, s1_t = [], []
                for i in range(NT):
                    sp = psum.tile([128, N], f32, tag="strip", name=f"sp{b}_{i}")
                    for k in range(KT):
                        lhsT = xt_t[k][:, i * 128 : (i + 1) * 128]
                        for j in range(N // NBLK):
                            nc.tensor.matmul(
                                sp[:, j * NBLK : (j + 1) * NBLK],
                                lhsT,
                                yt_t[k][:, j * NBLK : (j + 1) * NBLK],
                                start=(k == 0),
                                stop=(k == KT - 1),
                            )
                    ei = epool.tile([128, N], bf16, tag="e", name=f"es{b}_{i}")
                    si = stats.tile([128, 1], f32, tag="stats", name=f"s1_{b}_{i}")
                    nc.scalar.activation(
                        out=ei[:],
                        in_=sp[:],
                        func=mybir.ActivationFunctionType.Exp,
                        bias=bias_t[:],
                        scale=1.0,
                        accum_out=si[:],
                    )
                    es_t.append(ei)
                    s1_t.append(si)

                # natural-layout bf16 tiles (phase C operands)
                xn_t, yn_t = [], []
                for i in range(NT):
                    xi = nat.tile([128, D], bf16, tag="nat", name=f"xn{b}_{i}")
                    yi = nat.tile([128, D], bf16, tag="nat", name=f"yn{b}_{i}")
                    nc.sync.dma_start(xi[:], xn_d.ap()[b, i * 128 : (i + 1) * 128, :])
                    nc.sync.dma_start(yi[:], yn_d.ap()[b, i * 128 : (i + 1) * 128, :])
                    xn_t.append(xi)
                    yn_t.append(yi)

                # ---- phase B: transpose E_S -> E_T (8 tiles per PSUM bank) ----
                et_t, s2_t = [], []
                for j in range(NT):
                    ej = epool.tile([128, N], bf16, tag="e", name=f"et{b}_{j}")
                    parts = []
                    for half in range(2):
                        tp = psum.tile(
                            [128, 1024], bf16, tag="strip", name=f"tp{b}_{j}_{half}"
                        )
                        for u in range(8):
                            i = half * 8 + u
                            nc.tensor.matmul(
                                tp[:, u * 128 : (u + 1) * 128],
                                es_t[i][:, j * 128 : (j + 1) * 128],
                                ident[:],
                                is_transpose=True,
                                start=True,
                                stop=True,
                                skip_group_check=True,
                            )
                        ph = stats.tile(
                            [128, 1], f32, tag="stats", name=f"s2p_{b}_{j}_{half}"
                        )
                        nc.vector.tensor_scalar(
                            out=ej[:, half * 1024 : (half + 1) * 1024],
                            in0=tp[:],
                            scalar1=0.0,
                            scalar2=0.0,
                            op0=mybir.AluOpType.add,
                            op1=mybir.AluOpType.add,
                            accum_out=ph[:],
                        )
                        parts.append(ph)
                    sj = stats.tile([128, 1], f32, tag="stats", name=f"s2_{b}_{j}")
                    nc.vector.tensor_add(sj[:], parts[0][:], parts[1][:])
                    et_t.append(ej)
                    s2_t.append(sj)

                # ---- phase C: PV matmuls + epilogue ----
                for i in range(NT):
                    for which in range(2):  # 0: A1 (rows i*128), 1: A2 (rows N+i*128)
                        et = et_t if which == 0 else es_t
                        rn = yn_t if which == 0 else xn_t
                        mult_n = xn_t[i] if which == 0 else yn_t[i]
                        si = (s1_t if which == 0 else s2_t)[i]
                        op = psum.tile(
                            [128, D], f32, tag="strip", name=f"o{b}_{i}_{which}"
                        )
                        for m in range(NT):
                            nc.tensor.matmul(
                                op[:],
                                et[m][:, i * 128 : (i + 1) * 128],
                                rn[m][:],
                                start=(m == 0),
                                stop=(m == NT - 1),
                            )
                        ri = stats.tile(
                            [128, 1], f32, tag="stats", name=f"r{b}_{i}_{which}"
                        )
                        nc.vector.reciprocal(ri[:], si[:])
                        ai = abuf.tile([128, D], f32, tag="a", name=f"a{b}_{i}_{which}")
                        nc.vector.tensor_mul(ai[:], op[:], mult_n[:])
                        nc.vector.tensor_scalar_mul(ai[:], ai[:], ri[:])
                        row0 = which * N + i * 128
                        nc.sync.dma_start(out_d.ap()[b, row0 : row0 + 128, :], ai[:])

    nc.compile()
    return nc


def _prep(arr_f32: np.ndarray) -> tuple[np.ndarray, np.ndarray]:
    """arr [Bc, N, D] f32 -> (k-tiled transpose f32 [Bc,KT,128,N], bf16 natural)."""
    bc = arr_f32.shape[0]
    t = np.zeros((bc, KT * 128, N), np.float32)
    t[:, :D, :] = arr_f32.transpose(0, 2, 1)
    t = t.reshape(bc, KT, 128, N)
    return np.ascontiguousarray(t), np.ascontiguousarray(arr_f32.astype(_BF16))


def kernel(X, Y, _trace=False, _trace_kwargs=None):
    from concourse.bass_utils import run_bass_kernel_spmd

    X = np.asarray(X, dtype=np.float32)
    Y = np.asarray(Y, dtype=np.float32)
    assert X.shape == (B, N, D) and Y.shape == (B, N, D)

    if "nc" not in _cache:
        _cache["nc"] = _build()
    nc = _cache["nc"]

    in_maps = []
    for c in range(NCORES):
        sl = slice(c * BPC, (c + 1) * BPC)
        xt, xn = _prep(X[sl])
        yt, yn = _prep(Y[sl])
        in_maps.append({"xt": xt, "yt": yt, "xn": xn, "yn": yn})

    res = run_bass_kernel_spmd(
        nc,
        in_maps,
        core_ids=list(range(NCORES)),
        trace=_trace,
        **(_trace_kwargs or {}),
    )
    _cache["last_results"] = res

    out = np.empty((B, 2 * N, D), np.float32)
    for c in range(NCORES):
        out[c * BPC : (c + 1) * BPC] = res.results[c]["out"]
    return out


# revision 5
# speedup vs baseline: 1.1532x; 1.1307x over previous
"""Bidirectional co-attention kernel for Trainium2 (8 NeuronCores).

Problem: X, Y: (16, 2048, 300) f32.
  S_b = X_b @ Y_b^T                      (2048 x 2048 per batch)
  A1 = softmax_rows(S) @ Y * X
  A2 = softmax_rows(S^T) @ X * Y
  out = concat(A1, A2, axis=1)           -> (16, 4096, 300) f32

Sharding: data-parallel over batch, 2 batches per core, no cross-core comm.

Per-core algorithm (per batch):
  Phase A: S strips [128n x 2048m] on TensorE in float32r (full-rate,
    ~11-bit-mantissa accurate).  E_S = exp(S - 100) on ScalarE straight out
    of PSUM into bf16 SBUF; accum_out gives row sums s1[n] for free.
    (Fixed shift instead of row-max: scores are N(0, 300) so max |S| ~ 95;
    exp(S-100) never overflows and row maxes are far above the underflow
    cliff.  Normalization cancels the shift exactly.)
  Phase B: E_T = E_S^T via PE-mode transposes (128x128 bf16), 8 tiles packed
    per PSUM bank, evicted by VectorE tensor_scalar copy whose accum_out
    yields the column sums s2[m].
  Phase C: O1[n,:] = sum_m E_T[m, n-slice]^T @ Y[m,:] (bf16 matmul, f32 PSUM
    accum), A1 = O1 * X * (1/s1) on VectorE.  Direction 2 symmetric from E_S.
"""

import numpy as np
import ml_dtypes

B, N, D = 16, 2048, 300
NCORES = 8
BPC = B // NCORES  # batches per core
NT = N // 128  # 16 row-tiles
KT = 3  # k-tiles over D (128+128+44, zero-padded to 384)
GSHIFT = -100.0
DP = 304  # natural-layout tiles padded: col 300 = 1.0 (softmax denominator trick)
NBLK = 512  # moving-dim block for the score matmuls

_BF16 = ml_dtypes.bfloat16

_cache: dict[str, object] = {}


def _build():
    import concourse.bacc as bacc
    import concourse.mybir as mybir
    import concourse.tile as tile
    from concourse.masks import make_identity

    nc = bacc.Bacc("TRN2", target_bir_lowering=False, debug=False, num_devices=NCORES)

    f32 = mybir.dt.float32
    f32r = mybir.dt.float32r
    bf16 = mybir.dt.bfloat16

    xt_d = nc.dram_tensor("xt", [BPC, KT, 128, N], f32r, kind="ExternalInput")
    yt_d = nc.dram_tensor("yt", [BPC, KT, 128, N], f32r, kind="ExternalInput")
    xn_d = nc.dram_tensor("xn", [BPC, N, DP], bf16, kind="ExternalInput")
    yn_d = nc.dram_tensor("yn", [BPC, N, DP], bf16, kind="ExternalInput")
    out_d = nc.dram_tensor("out", [BPC, 2 * N, D], f32, kind="ExternalOutput")

    with tile.TileContext(nc) as tc:
        with (
            tc.tile_pool(name="const", bufs=1) as constp,
            tc.tile_pool(name="io", bufs=KT) as io,
            tc.tile_pool(name="nat", bufs=2 * NT) as nat,
            tc.tile_pool(name="epool", bufs=2 * NT) as epool,
            tc.tile_pool(name="stats", bufs=8 * NT) as stats,
            tc.tile_pool(name="abuf", bufs=4) as abuf,
            tc.tile_pool(name="psum", bufs=2, space="PSUM") as psum,
        ):
            bias_t = constp.tile([128, 1], f32, name="bias_t")
            nc.vector.memset(bias_t[:], GSHIFT)
            ident = constp.tile([128, 128], bf16, name="ident")
            make_identity(nc, ident[:])

            for b in range(BPC):
                xt_t, yt_t = [], []
                for k in range(KT):
                    xk = io.tile([128, N], f32r, tag="xt", name=f"xt{b}_{k}")
                    yk = io.tile([128, N], f32r, tag="yt", name=f"yt{b}_{k}")
                    nc.sync.dma_start(yk[:], yt_d.ap()[b, k])
                    for c in range(4):
                        nc.sync.dma_start(
                            xk[:, c * 512 : (c + 1) * 512],
                            xt_d.ap()[b, k, :, c * 512 : (c + 1) * 512],
                        )
                    xt_t.append(xk)
                    yt_t.append(yk)

                # ---- phase A: S strips + exp ----
                es_t = []
                for i in range(NT):
                    sp = psum.tile([128, N], f32, tag="strip", name=f"sp{b}_{i}")
                    for k in range(KT):
                        lhsT = xt_t[k][:, i * 128 : (i + 1) * 128]
                        for j in range(N // NBLK):
                            nc.tensor.matmul(
                                sp[:, j * NBLK : (j + 1) * NBLK],
                                lhsT,
                                yt_t[k][:, j * NBLK : (j + 1) * NBLK],
                                start=(k == 0),
                                stop=(k == KT - 1),
                            )
                    ei = epool.tile([128, N], bf16, tag="e", name=f"es{b}_{i}")
                    nc.scalar.activation(
                        out=ei[:],
                        in_=sp[:],
                        func=mybir.ActivationFunctionType.Exp,
                        bias=bias_t[:],
                        scale=1.0,
                    )
                    es_t.append(ei)

                # natural-layout bf16 tiles (phase C operands)
                xn_t, yn_t = [], []
                for i in range(NT):
                    xi = nat.tile([128, DP], bf16, tag="nat", name=f"xn{b}_{i}")
                    yi = nat.tile([128, DP], bf16, tag="nat", name=f"yn{b}_{i}")
                    nc.sync.dma_start(xi[:], xn_d.ap()[b, i * 128 : (i + 1) * 128, :])
                    nc.sync.dma_start(yi[:], yn_d.ap()[b, i * 128 : (i + 1) * 128, :])
                    xn_t.append(xi)
                    yn_t.append(yi)

                # ---- phase B: transpose E_S -> E_T (8 tiles per PSUM bank) ----
                et_t = []
                for j in range(NT):
                    ej = epool.tile([128, N], bf16, tag="e", name=f"et{b}_{j}")
                    for half in range(2):
                        tp = psum.tile(
                            [128, 1024], bf16, tag="strip", name=f"tp{b}_{j}_{half}"
                        )
                        for u in range(8):
                            i = half * 8 + u
                            nc.tensor.matmul(
                                tp[:, u * 128 : (u + 1) * 128],
                                es_t[i][:, j * 128 : (j + 1) * 128],
                                ident[:],
                                is_transpose=True,
                                start=True,
                                stop=True,
                                skip_group_check=True,
                            )
                        nc.vector.tensor_copy(
                            ej[:, half * 1024 : (half + 1) * 1024], tp[:]
                        )
                    et_t.append(ej)

                # ---- phase C: PV matmuls + epilogue ----
                for i in range(NT):
                    for which in range(2):  # 0: A1 (rows i*128), 1: A2 (rows N+i*128)
                        et = et_t if which == 0 else es_t
                        rn = yn_t if which == 0 else xn_t
                        mult_n = xn_t[i] if which == 0 else yn_t[i]
                        op = psum.tile(
                            [128, D + 1], f32, tag="strip", name=f"o{b}_{i}_{which}"
                        )
                        for m in range(NT):
                            nc.tensor.matmul(
                                op[:],
                                et[m][:, i * 128 : (i + 1) * 128],
                                rn[m][:, : D + 1],
                                start=(m == 0),
                                stop=(m == NT - 1),
                            )
                        ri = stats.tile(
                            [128, 1], f32, tag="stats", name=f"r{b}_{i}_{which}"
                        )
                        nc.vector.reciprocal(ri[:], op[:, D : D + 1])
                        ai = abuf.tile([128, D], f32, tag="a", name=f"a{b}_{i}_{which}")
                        nc.vector.tensor_mul(ai[:], op[:, :D], mult_n[:, :D])
                        nc.vector.tensor_scalar_mul(ai[:], ai[:], ri[:])
                        row0 = which * N + i * 128
                        nc.sync.dma_start(out_d.ap()[b, row0 : row0 + 128, :], ai[:])

    nc.compile()
    return nc


def _prep(arr_f32: np.ndarray) -> tuple[np.ndarray, np.ndarray]:
    """arr [Bc, N, D] f32 -> (k-tiled transpose f32 [Bc,KT,128,N],
    bf16 natural [Bc, N, DP] with a ones column at index D)."""
    bc = arr_f32.shape[0]
    t = np.zeros((bc, KT * 128, N), np.float32)
    t[:, :D, :] = arr_f32.transpose(0, 2, 1)
    t = t.reshape(bc, KT, 128, N)
    nat = np.zeros((bc, N, DP), _BF16)
    nat[:, :, :D] = arr_f32
    nat[:, :, D] = 1.0
    return np.ascontiguousarray(t), nat


def kernel(X, Y, _trace=False, _trace_kwargs=None):
    from concourse.bass_utils import run_bass_kernel_spmd

    X = np.asarray(X, dtype=np.float32)
    Y = np.asarray(Y, dtype=np.float32)
    assert X.shape == (B, N, D) and Y.shape == (B, N, D)

    if "nc" not in _cache:
        _cache["nc"] = _build()
    nc = _cache["nc"]

    in_maps = []
    for c in range(NCORES):
        sl = slice(c * BPC, (c + 1) * BPC)
        xt, xn = _prep(X[sl])
        yt, yn = _prep(Y[sl])
        in_maps.append({"xt": xt, "yt": yt, "xn": xn, "yn": yn})

    res = run_bass_kernel_spmd(
        nc,
        in_maps,
        core_ids=list(range(NCORES)),
        trace=_trace,
        **(_trace_kwargs or {}),
    )
    _cache["last_results"] = res

    out = np.empty((B, 2 * N, D), np.float32)
    for c in range(NCORES):
        out[c * BPC : (c + 1) * BPC] = res.results[c]["out"]
    return out
